# revision 4
# baseline (speedup 1.0000x reference)
"""Trainium2 Bass kernel for nn_ReallocationMapEncoder.

The reference network is three NAC layers (y = x @ (tanh(W_hat)*sigmoid(M_hat)).T)
applied to a [nsteps, nsyms, nsyms, 3] grid of normalized (t, a, b) indices,
plus a gb broadcast on the trailing axis. NAC is linear in x, so the whole
network collapses to one effective matrix Weff = W3 @ W2 @ W1 of shape [2, 3]:

    y[t, a, b, c] = gb[c] + (t/2)*Weff[c,0] + (a/2048)*Weff[c,1] + (b/2048)*Weff[c,2]

The output [2, 2048, 2048, 2] f32 (67 MB) is a separable affine ramp; the kernel
is purely output-write-bandwidth bound (memory regime).

Device strategy (8 cores, data-parallel on the `a` axis, 256 rows each, so each
core writes 8.4 MB): a DVE iota builds J[p, j] = j; every output chunk
[128 a-rows, fsz free elems at c-stride] is a fused DVE tensor_scalar

    out[p, b, c] = J[p, b] * (Weff[c,2]/nsyms) + bias[p, (t,blk,c)]

where bias (a tiny [128, 8] per-core input) folds the gb/t/a terms.

Perf structure (verified against the CoreSim cost model):
- All DMAs are HWDGE on the SP engine (nc.sync). SWDGE (gpsimd) holds the
  Pool engine through descriptor generation and serializes; HWDGE frees the
  sequencer before the transfer, so back-to-back output DMAs pack the SDMA
  engines wall-to-wall (the 360 GB/s stream is the roofline: 8.4 MB -> 23.3 us).
- The bias DMA's end-to-end latency (~2.4 us: hwdge + dge delay + sem prop)
  is the unavoidable head of the critical path; the J iotas run on Pool in
  parallel with it.
- Output is cut into chunks, small first (256, 1024, 2816 free elems, then
  three full 4096 tiles): the first DMA issues ~2.8 us in, and the DVE stays
  ahead of the DMA stream even if strided writes fall to 1x mode on HW.
- Every chunk gets its own SBUF buffer (no slot reuse -> no WAR waits); each
  output DMA carries exactly one wait (DVE sem), fitting walrus's HWDGE
  single-wait slot.

Sync-wait slot limits in walrus codegen (HWDGE DMA: 1, DVE/ACT: 2):
_legalize_waits splits any over-limit instruction (the Tile kernel-tail
drain) into single-wait Drain carriers.
"""

import numpy as np

NSTEPS = 2
NSYMS = 2048
NCORES = 8
A_PER_CORE = NSYMS // NCORES          # 256
BLKS = A_PER_CORE // 128              # 2 partition blocks per core
F = NSYMS * 2                         # 4096 free elements per a-row (b, c interleaved)

# Per-tile f-split sizes, applied tile-by-tile in (t, blk) order: small chunks
# first so the DMA stream starts early; all bounds even (c-pairs).
CHUNK_SIZES = [256, 1024, 2816, 4096, 4096, 4096]

# J iota pieces (b ranges) sized so each chunk's J slice is ready before the
# bias DMA lands.
J_SPLITS = [(0, 128), (128, 640), (640, 2048)]

# Engines (round-robin) that issue the output DMAs; both SP ("sync") and ACT
# ("scalar") have HWDGE rings, and alternating engines lets DMA k+1's
# issue/wait overlap DMA k's transfer.
OUT_DMA_ENGINES = ["sync", "scalar"]

# Engine for the bias input DMA.
BIAS_DMA_ENGINE = "sync"


def _chunks():
    tiles = [(t, blk) for t in range(NSTEPS) for blk in range(BLKS)]
    out, ti, f = [], 0, 0
    for sz in CHUNK_SIZES:
        t, blk = tiles[ti]
        out.append((t, blk, f, f + sz))
        f += sz
        if f == F:
            ti, f = ti + 1, 0
    assert ti == len(tiles) and f == 0, "CHUNK_SIZES must tile 4 x F exactly"
    return out

_CACHE = {}


def _build_bass(scales):
    import concourse.bass as bass
    import concourse.mybir as mybir
    from concourse.tile import TileContext

    f32 = mybir.dt.float32
    nc = bass.Bass(trn_type="TRN2")

    bias_in = nc.dram_tensor("bias_in", [128, NSTEPS * BLKS * 2], f32, kind="ExternalInput")
    out = nc.dram_tensor("out", [NSTEPS, BLKS, 128, F], f32, kind="ExternalOutput")

    chunks = _chunks()
    with TileContext(nc) as tc:
        with (
            tc.tile_pool(name="const", bufs=1) as const,
            tc.tile_pool(name="outp", bufs=len(chunks)) as outp,
        ):
            bias_sb = const.tile([128, NSTEPS * BLKS * 2], f32)
            getattr(nc, BIAS_DMA_ENGINE).dma_start(bias_sb[:], bias_in[:])

            J = const.tile([128, NSYMS], f32)
            for b0, b1 in J_SPLITS:
                nc.gpsimd.iota(
                    J[:, b0:b1], pattern=[[1, b1 - b0]], base=b0,
                    channel_multiplier=0, allow_small_or_imprecise_dtypes=True,
                )

            for k, (t, blk, f0, f1) in enumerate(chunks):
                fsz = f1 - f0
                ot = outp.tile([128, fsz], f32)
                otv = ot[:].rearrange("p (b c) -> p b c", c=2)
                for c in range(2):
                    idx = (t * BLKS + blk) * 2 + c
                    nc.vector.tensor_scalar(
                        otv[:, :, c],
                        J[:, f0 // 2 : f1 // 2],
                        scales[c],
                        bias_sb[:, idx : idx + 1],
                        mybir.AluOpType.mult,
                        mybir.AluOpType.add,
                    )
                eng = OUT_DMA_ENGINES[k % len(OUT_DMA_ENGINES)]
                getattr(nc, eng).dma_start(out[t, blk, :, f0:f1], ot[:])

    _legalize_waits(nc, mybir)
    return nc


def _legalize_waits(nc, mybir):
    """This walrus build fits very few semaphore waits per instruction (one
    for most engine structs). Tile's auto-generated kernel-tail drain waits
    on every DMA lane + engine sem at once; split any multi-wait instruction
    into a chain of single-wait Drain carriers on the same engine."""
    for func in nc.m.functions:
        for block in func.blocks:
            insts = list(block.instructions)
            new_insts = []
            changed = False
            for inst in insts:
                si = inst.sync_info
                waits = list(si.on_wait) if si is not None and si.on_wait else []
                if len(waits) > 1:
                    for w in waits[:-1]:
                        d = mybir.InstDrain(
                            name=f"{inst.name}-waitsplit-{len(new_insts)}",
                            ins=[],
                            outs=[],
                            bass_is_fusable=False,
                        )
                        d.engine = inst.engine
                        d.sync_info = mybir.SyncInfo(on_wait=[w], on_update=[])
                        new_insts.append(d)
                    inst.sync_info = mybir.SyncInfo(
                        on_wait=[waits[-1]], on_update=list(si.on_update or [])
                    )
                    changed = True
                new_insts.append(inst)
            if changed:
                block.instructions = new_insts


def _host_consts(gb, w_hat1, m_hat1, w_hat2, m_hat2, w_hat3, m_hat3):
    def nacw(w, m):
        w = np.asarray(w, np.float64)
        m = np.asarray(m, np.float64)
        return np.tanh(w) * (1.0 / (1.0 + np.exp(-m)))

    weff = nacw(w_hat3, m_hat3) @ nacw(w_hat2, m_hat2) @ nacw(w_hat1, m_hat1)  # [2,3]
    gb = np.asarray(gb, np.float64)

    scales = [float(np.float32(weff[c, 2] / NSYMS)) for c in range(2)]

    # bias[core][p, (t,blk,c)] = gb[c] + (t/2)Weff[c,0] + (a/2048)Weff[c,1]
    biases = []
    for core in range(NCORES):
        bias = np.empty((128, NSTEPS, BLKS, 2), np.float64)
        for t in range(NSTEPS):
            for blk in range(BLKS):
                a = (core * A_PER_CORE + blk * 128 + np.arange(128)) / NSYMS
                for c in range(2):
                    bias[:, t, blk, c] = (
                        gb[c] + (t / NSTEPS) * weff[c, 0] + a * weff[c, 1]
                    )
        biases.append(np.ascontiguousarray(bias.reshape(128, -1), np.float32))
    return scales, biases


def kernel(market, gb, w_hat1, m_hat1, w_hat2, m_hat2, w_hat3, m_hat3):
    from concourse.bass_utils import run_bass_kernel_spmd

    scales, biases = _host_consts(gb, w_hat1, m_hat1, w_hat2, m_hat2, w_hat3, m_hat3)
    # the tensor_scalar immediates (scales) are baked into the traced program,
    # so the compiled module is keyed on them
    key = ("nc", tuple(scales))
    if key not in _CACHE:
        _CACHE[key] = _build_bass(scales)
    nc = _CACHE[key]
    _CACHE["last_nc"] = nc

    in_maps = [{"bias_in": biases[core]} for core in range(NCORES)]
    res = run_bass_kernel_spmd(nc, in_maps, core_ids=list(range(NCORES)))
    parts = [r["out"].reshape(NSTEPS, A_PER_CORE, NSYMS, 2) for r in res.results]
    return np.concatenate(parts, axis=1)
